# revision 19
# baseline (speedup 1.0000x reference)
"""EnhancedTemporalAttention Trainium2 kernel.

Full module: GroupNorm(32) -> QKV 1x1conv -> 8-head attention (softmax) ->
out 1x1conv + bias -> +residual, on x [4, 512, 2048] fp32.

Sharding: 8 cores = (batch b = core//2) x (query half = core%2).  Each core
computes GroupNorm + K/V projections over the full sequence for its batch
(duplicated across the pair), Q projection + attention + out projection for
its 1024-query half.  Output slices are disjoint; host just concatenates.

All matmuls run as float32r (fp32 storage, reduced-precision multiply at
full PE rate).  Attention uses the transposed-scores layout (keys on
partitions); softmax denominators ride as a 65th ones-row on the V^T
stationary operand; exp runs on ScalarE straight out of PSUM.  The first
head-pair's attention is interleaved with the V^T projection so ScalarE
starts its exp stream early.
"""
import sys

sys.path.insert(0, "/opt/trn_rl_repo")

import numpy as np

import concourse.bacc as bacc
import concourse.bass as bass
import concourse.tile as tile
from concourse import mybir
from concourse.bass_utils import run_bass_kernel_spmd

F32 = mybir.dt.float32
F32R = mybir.dt.float32r

B = 4
C = 512
N = 2048
NQ = 1024          # queries per core
H = 8
D = 64
G = 32             # groupnorm groups
CPG = C // G       # 16 channels per group
EPS = 1e-4
SCALE = D ** -0.5
NT = C // 128      # 4 channel tiles
NKB = N // 128     # 16 key blocks
AF = mybir.ActivationFunctionType
ALU = mybir.AluOpType

# (j, h) chunk sequence in groups of 3 (one exp instruction each)
CHUNKS = [(j, h) for j in range(NKB) for h in range(2)]
GROUPS = [CHUNKS[i:i + 3] for i in range(0, len(CHUNKS), 3)]


def _build(taps=False):
    nc = bacc.Bacc("TRN2", target_bir_lowering=False, debug=False)
    x_in = nc.dram_tensor("x", [C, N], F32, kind="ExternalInput").ap()
    xq_in = nc.dram_tensor("xq", [C, NQ], F32, kind="ExternalInput").ap()
    wqkvT_in = nc.dram_tensor("wqkvT", [C, 3 * C], F32, kind="ExternalInput").ap()
    woutT_in = nc.dram_tensor("woutT", [C, C], F32, kind="ExternalInput").ap()
    gbo_in = nc.dram_tensor("gbo", [C, 3], F32, kind="ExternalInput").ap()
    gblk_in = nc.dram_tensor("gblk", [128, 8], F32, kind="ExternalInput").ap()
    gbt_in = nc.dram_tensor("gbt", [8, 128], F32, kind="ExternalInput").ap()
    y_out = nc.dram_tensor("y", [C, NQ], F32, kind="ExternalOutput").ap()
    # scratch
    kind_t = "ExternalOutput" if taps else "Internal"
    mr_d = nc.dram_tensor("mr_d", [C, 2], F32, kind=kind_t).ap()
    den_d = nc.dram_tensor("den_d", [H * NQ], F32, kind=kind_t).ap()
    den2_d = nc.dram_tensor("den2_d", [H * NQ], F32).ap()
    tap = {}
    if taps:
        for nm, shp in (("t_xn0", [128, N]), ("t_o65a", [128, 512]),
                        ("t_o65b", [128, 512]), ("t_op00", [128, 512])):
            tap[nm] = nc.dram_tensor(nm, shp, F32, kind="ExternalOutput").ap()

    from contextlib import ExitStack
    with tile.TileContext(nc) as tc, ExitStack() as ctx:
        persist = ctx.enter_context(tc.tile_pool(name="persist", bufs=1))
        gn = ctx.enter_context(tc.tile_pool(name="gn", bufs=1))
        pspool = ctx.enter_context(tc.tile_pool(name="ps", bufs=1,
                                                space="PSUM"))
        expp = ctx.enter_context(tc.tile_pool(name="expp", bufs=2))
        o65p = ctx.enter_context(tc.tile_pool(name="o65p", bufs=4))

        # persistent activation tensors
        q_sb = [persist.tile([128, NQ], F32R, tag=f"q{m}", name=f"q{m}")
                for m in range(NT)]
        k_sb = [persist.tile([128, N], F32R, tag=f"k{m}", name=f"k{m}")
                for m in range(NT)]
        vT_sb = [persist.tile([128, H, D + 1], F32R, tag=f"vT{nb}",
                              name=f"vT{nb}") for nb in range(NKB)]

        den_r = den_d.rearrange("(a b) -> a b", b=512)

        def new_S():
            return pspool.tile([128, 3, 512], F32, tag="S", name="S",
                               bufs=2)

        def attn_group(qc, m, av, grp_chunks):
            ng = len(grp_chunks)
            psc = new_S()
            for i, (j, h) in enumerate(grp_chunks):
                nc.tensor.matmul(
                    psc[:, i, :],
                    lhsT=k_sb[m][h * D:(h + 1) * D, j * 128:(j + 1) * 128],
                    rhs=q_sb[m][h * D:(h + 1) * D,
                                qc * 512:(qc + 1) * 512],
                    start=True, stop=True, tile_position=(h * D, 0))
            eT = expp.tile([128, 3, 512], F32R, tag="e", name="e")
            nc.scalar.activation(out=eT[:, 0:ng, :], in_=psc[:, 0:ng, :],
                                 func=AF.Exp, scale=SCALE)
            for i, (j, h) in enumerate(grp_chunks):
                nc.tensor.matmul(
                    av[h], lhsT=vT_sb[j][:, 2 * m + h, :], rhs=eT[:, i, :],
                    start=(j == 0), stop=(j == NKB - 1))

        def pair_drain(qc, m, av):
            o65s = []
            for h in range(2):
                o65 = o65p.tile([128, 512], F32, tag="o65", name="o65")
                nc.vector.tensor_copy(o65[0:D + 1, :], av[h][0:D + 1, :])
                if taps and m == 0 and qc == 0:
                    nc.sync.dma_start(
                        out=tap["t_o65a" if h == 0 else "t_o65b"], in_=o65)
                nc.scalar.dma_start(out=den_r[qc * 8 + m * 2 + h, :],
                                    in_=o65[D:D + 1, :])
                o65s.append(o65)
            return o65s

        with tc.tile_pool(name="xpool", bufs=1) as xpool, \
             tc.tile_pool(name="xnpool", bufs=1) as xnpool, \
             tc.tile_pool(name="wq", bufs=1) as wqp:
            # ---- input loads, critical-path first: x (in 512-col chunks
            # so bn_stats pipelines), then qkv weights ----
            X = []    # f32r storage; XF = f32 views for DVE reads
            for t in range(NT):
                xt = xpool.tile([128, N], F32R, tag=f"X{t}", name=f"X{t}")
                for sg in range(4):
                    nc.sync.dma_start(
                        out=xt[:, sg * 512:(sg + 1) * 512],
                        in_=x_in[t * 128:(t + 1) * 128,
                                 sg * 512:(sg + 1) * 512].bitcast(F32R))
                X.append(xt)
            XF = [xt.bitcast(F32) for xt in X]
            # ACT table preload off the critical path (Sqrt now; Exp is
            # chained after the real Sqrt below so it can't evict it early)
            eps_t = gn.tile([G, 1], F32, tag="eps_t")
            nc.vector.memset(eps_t, EPS)
            sqw = gn.tile([G, 1], F32, tag="sqw")
            nc.scalar.activation(out=sqw, in_=eps_t, func=AF.Sqrt)
            gblk = gn.tile([128, 8], F32R, tag="gblk")
            nc.sync.dma_start(out=gblk, in_=gblk_in.bitcast(F32R))
            gbt = gn.tile([8, 128], F32R, tag="gbt")
            nc.sync.dma_start(out=gbt, in_=gbt_in.bitcast(F32R))
            XQ = []
            gbo = []
            for t in range(NT):
                xqt = persist.tile([128, NQ], F32, tag=f"XQ{t}",
                                   name=f"XQ{t}")
                nc.sync.dma_start(out=xqt,
                                  in_=xq_in[t * 128:(t + 1) * 128, :])
                XQ.append(xqt)
                gt = persist.tile([128, 3], F32, tag=f"gbo{t}",
                                  name=f"gbo{t}")
                nc.sync.dma_start(out=gt,
                                  in_=gbo_in[t * 128:(t + 1) * 128, :])
                gbo.append(gt)
            wT = [wqp.tile([128, 3 * C], F32R, tag=f"wT{kc}",
                           name=f"wT{kc}") for kc in range(NT)]
            for sl in (1, 0, 2):   # k first (k-proj is emitted first)
                for kc in range(NT):
                    nc.sync.dma_start(
                        out=wT[kc][:, sl * C:(sl + 1) * C],
                        in_=wqkvT_in[kc * 128:(kc + 1) * 128,
                                     sl * C:(sl + 1) * C].bitcast(F32R))

            # ---- GroupNorm stats: bn_stats -> per-channel (mean, E[x^2])
            # -> PE block-ones matmul reduces 16-channel groups ----
            mvv = []
            for t in range(NT):
                stats = gn.tile([128, 4, 6], F32, tag=f"st{t}",
                                name=f"st{t}")
                for sg in range(4):
                    nc.vector.bn_stats(out=stats[:, sg, :],
                                       in_=XF[t][:, sg * 512:(sg + 1) * 512])
                mv = gn.tile([128, 2], F32, tag=f"mv{t}", name=f"mv{t}")
                nc.vector.bn_aggr(out=mv, in_=stats)
                mt = gn.tile([128, 2], F32R, tag=f"mvv{t}", name=f"mvv{t}")
                nc.vector.tensor_copy(mt[:, 0:1], mv[:, 0:1])
                sqm = gn.tile([128, 1], F32, tag=f"sqm{t}", name=f"sqm{t}")
                # E[x^2] = var + mean^2
                nc.vector.tensor_mul(sqm, mv[:, 0:1], mv[:, 0:1])
                nc.vector.tensor_tensor(out=mt[:, 1:2], in0=mv[:, 1:2],
                                        in1=sqm, op=ALU.add)
                mvv.append(mt)
            g8ps = new_S()     # group sums land in psum bank 0, [8, 8]
            for t in range(NT):
                nc.tensor.matmul(g8ps[0:8, 0, t * 2:(t + 1) * 2],
                                 lhsT=gblk, rhs=mvv[t],
                                 start=(t == 0), stop=(t == NT - 1),
                                 skip_group_check=True)
            g8 = gn.tile([8, NT, 2], F32, tag="g8")
            nc.vector.tensor_copy(g8.rearrange("p t s -> p (t s)"),
                                  g8ps[0:8, 0, 0:8])
            mean8 = gn.tile([8, NT], F32, tag="mean8")
            nc.vector.tensor_scalar_mul(mean8, g8[:, :, 0], 1.0 / CPG)
            ex28 = gn.tile([8, NT], F32, tag="ex28")
            nc.vector.tensor_scalar_mul(ex28, g8[:, :, 1], 1.0 / CPG)
            msq8 = gn.tile([8, NT], F32, tag="msq8")
            nc.vector.tensor_mul(msq8, mean8, mean8)
            var8 = gn.tile([8, NT], F32, tag="var8")
            nc.vector.tensor_tensor(out=var8, in0=ex28, in1=msq8,
                                    op=ALU.subtract)
            std8 = gn.tile([8, NT], F32, tag="std8")
            nc.scalar.activation(out=std8, in_=var8, func=AF.Sqrt,
                                 bias=eps_t[0:8, :])
            rstd8 = gn.tile([8, NT], F32, tag="rstd8")
            nc.vector.reciprocal(rstd8, std8)
            # preload the Exp table now; input std8 forces it after Sqrt
            warm = gn.tile([8, NT], F32, tag="warm")
            nc.scalar.activation(out=warm, in_=std8, func=AF.Exp)
            mr8 = gn.tile([8, NT, 2], F32R, tag="mr8")
            nc.vector.tensor_copy(mr8[:, :, 0:1],
                                  mean8.rearrange("p (t o) -> p t o", o=1))
            nc.vector.tensor_copy(mr8[:, :, 1:2],
                                  rstd8.rearrange("p (t o) -> p t o", o=1))
            # broadcast group stats to channels via a K=8 ones matmul
            msps = new_S()
            for t in range(NT):
                nc.tensor.matmul(msps[:, 0, t * 2:(t + 1) * 2],
                                 lhsT=gbt, rhs=mr8[:, t, :],
                                 start=(t == 0), stop=(t == NT - 1),
                                 skip_group_check=True)
            mscall = msps[:, 0, 0:2 * NT].rearrange("p (t s) -> p t s", s=2)

            # per-channel scale/bias, then normalize (in place over X)
            xnq = []
            for t in range(NT):
                scale_c = gn.tile([128, 1], F32, tag=f"sc{t}", name=f"sc{t}")
                nc.vector.tensor_mul(scale_c, mscall[:, t, 1:2],
                                     gbo[t][:, 0:1])
                tmp = gn.tile([128, 1], F32, tag=f"tmp{t}", name=f"tmp{t}")
                nc.vector.tensor_mul(tmp, mscall[:, t, 0:1], scale_c)
                bias_c = gn.tile([128, 1], F32, tag=f"bc{t}", name=f"bc{t}")
                nc.vector.tensor_tensor(out=bias_c, in0=gbo[t][:, 1:2],
                                        in1=tmp, op=ALU.subtract)
                xnt = X[t]
                nc.vector.tensor_scalar(out=xnt, in0=XF[t], scalar1=scale_c,
                                        scalar2=bias_c, op0=ALU.mult,
                                        op1=ALU.add)
                if taps and t == 0:
                    nc.sync.dma_start(out=tap["t_xn0"], in_=xnt.bitcast(F32))
                xnqt = xnpool.tile([128, NQ], F32R, tag=f"xnq{t}",
                                   name=f"xnq{t}")
                nc.vector.tensor_scalar(out=xnqt, in0=XQ[t], scalar1=scale_c,
                                        scalar2=bias_c, op0=ALU.mult,
                                        op1=ALU.add)
                xnq.append(xnqt)
            xn = X

            # ---------- QKV projections ----------
            ones_sb = gn.tile([128, H], F32, tag="ones_sb")
            nc.vector.memset(ones_sb, 1.0)
            # k: rows 512..1023, over full sequence
            for m in range(NT):
                for ncx in range(N // 512):
                    psy = new_S()
                    ps = psy[:, 0, :]
                    for kc in range(NT):
                        nc.tensor.matmul(
                            ps, lhsT=wT[kc][:, C + m * 128:C + (m + 1) * 128],
                            rhs=xn[kc][:, ncx * 512:(ncx + 1) * 512],
                            start=(kc == 0), stop=(kc == NT - 1))
                    nc.vector.tensor_copy(
                        k_sb[m][:, ncx * 512:(ncx + 1) * 512], ps)
            # q: rows 0..511 of qkv, over the core's query half
            for m in range(NT):
                for ncx in range(NQ // 512):
                    psy = new_S()
                    ps = psy[:, 0, :]
                    for kc in range(NT):
                        nc.tensor.matmul(
                            ps, lhsT=wT[kc][:, m * 128:(m + 1) * 128],
                            rhs=xnq[kc][:, ncx * 512:(ncx + 1) * 512],
                            start=(kc == 0), stop=(kc == NT - 1))
                    nc.vector.tensor_copy(
                        q_sb[m][:, ncx * 512:(ncx + 1) * 512], ps)
            # v^T interleaved with head-pair (0,0) attention so ScalarE's
            # exp stream starts while PE is still projecting
            av00 = [pspool.tile([D + 1, 512], F32, tag="av", name="av",
                                bufs=2) for _ in range(2)]
            started = [False]
            gi_by_maxj = {}
            for gi, gch in enumerate(GROUPS):
                mj = max(j for j, _ in gch)
                gi_by_maxj.setdefault(mj, []).append(gi)
            for nb in range(NKB):
                psy = new_S()
                ps = psy[:, 0, :]
                for kc in range(NT):
                    nc.tensor.matmul(
                        ps, lhsT=xn[kc][:, nb * 128:(nb + 1) * 128],
                        rhs=wT[kc][:, 2 * C:3 * C],
                        start=(kc == 0), stop=(kc == NT - 1))
                nc.vector.tensor_copy(
                    vT_sb[nb][:, :, 0:D],
                    ps.rearrange("p (h d) -> p h d", h=H))
                nc.vector.tensor_copy(
                    vT_sb[nb][:, :, D:D + 1],
                    ones_sb.rearrange("p (h o) -> p h o", o=1))
                for gi in gi_by_maxj.get(nb, []):
                    attn_group(0, 0, av00, GROUPS[gi])
            o65s00 = pair_drain(0, 0, av00)

        # ---------- rest of attention ----------
        with tc.tile_pool(name="opp", bufs=1) as opp, \
             tc.tile_pool(name="ytp", bufs=3) as ytp, \
             tc.tile_pool(name="rbp", bufs=6) as rbp:
            opair = {(kc, nck): opp.tile([128, 512], F32R,
                                         tag=f"op{kc}_{nck}",
                                         name=f"op{kc}_{nck}")
                     for kc in range(NT) for nck in range(2)}
            woutT = []
            for t in range(NT):
                wt = opp.tile([128, C], F32R, tag=f"woT{t}", name=f"woT{t}")
                nc.sync.dma_start(
                    out=wt,
                    in_=woutT_in[t * 128:(t + 1) * 128, :].bitcast(F32R))
                woutT.append(wt)

            def do_pair(qc, m):
                av = [pspool.tile([D + 1, 512], F32, tag="av", name="av",
                                  bufs=2) for _ in range(2)]
                for gch in GROUPS:
                    attn_group(qc, m, av, gch)
                return pair_drain(qc, m, av)

            def do_norm(qc, m, o65s):
                """reciprocal of the pair's dens (repacked [128,8]),
                broadcast back, normalize into opair[(m, qc)]."""
                r0 = qc * 8 + m * 2
                dpack = rbp.tile([128, 8], F32, tag="dpack", name="dpack")
                nc.scalar.dma_start(
                    out=dpack,
                    in_=den_d.rearrange("(a b) -> a b", b=8)[
                        r0 * 64:(r0 + 2) * 64, :])
                nc.vector.reciprocal(dpack, dpack)
                nc.scalar.dma_start(
                    out=den2_d.rearrange("(a b) -> a b", b=8)[
                        r0 * 64:(r0 + 2) * 64, :],
                    in_=dpack)
                op = opair[(m, qc)]
                # head 0: rb at partitions 0..63, multiply into opair rows
                rb = rbp.tile([D, 512], F32, tag="rb", name="rb")
                nc.scalar.dma_start(
                    out=rb,
                    in_=bass.AP(tensor=den2_d.tensor,
                                offset=den2_d.offset + r0 * 512,
                                ap=[[0, D], [1, 512]]))
                nc.vector.tensor_mul(op[0:D, :], o65s[0][0:D, :], rb)
                # head 1: shift raw rows into opair[64:128] while loading
                # its reciprocal bcast at partitions 64..127, then multiply
                # in place
                rb2 = rbp.tile([128, 512], F32, tag="rb2", name="rb2")
                nc.scalar.dma_start(
                    out=rb2[D:2 * D, :],
                    in_=bass.AP(tensor=den2_d.tensor,
                                offset=den2_d.offset + (r0 + 1) * 512,
                                ap=[[0, D], [1, 512]]))
                nc.scalar.dma_start(out=op[D:2 * D, :],
                                    in_=o65s[1][0:D, :].bitcast(F32R))
                nc.vector.tensor_mul(op[D:2 * D, :],
                                     op[D:2 * D, :].bitcast(F32),
                                     rb2[D:2 * D, :])

            def do_outproj(qc):
                for m2 in range(NT):
                    psy = new_S()
                    ps = psy[:, 0, :]
                    for kc in range(NT):
                        nc.tensor.matmul(
                            ps, lhsT=woutT[kc][:, m2 * 128:(m2 + 1) * 128],
                            rhs=opair[(kc, qc)],
                            start=(kc == 0), stop=(kc == NT - 1))
                    yt = ytp.tile([128, 512], F32, tag="yt", name="yt")
                    nc.vector.scalar_tensor_tensor(
                        out=yt, in0=ps, scalar=gbo[m2][:, 2:3],
                        in1=XQ[m2][:, qc * 512:(qc + 1) * 512],
                        op0=ALU.add, op1=ALU.add)
                    nc.sync.dma_start(
                        out=y_out[m2 * 128:(m2 + 1) * 128,
                                  qc * 512:(qc + 1) * 512],
                        in_=yt)

            # software-pipelined emission: out-proj of half 0 lands in the
            # middle of half 1's attention stream
            do_norm(0, 0, o65s00)
            for m in range(1, NT):
                do_norm(0, m, do_pair(0, m))
            for m in range(2):
                do_norm(1, m, do_pair(1, m))
            do_outproj(0)
            if taps:
                nc.sync.dma_start(out=tap["t_op00"],
                                  in_=opair[(0, 0)].bitcast(F32))
            for m in range(2, NT):
                do_norm(1, m, do_pair(1, m))
            do_outproj(1)

    nc.compile()
    return nc


_NC = None


def _get_nc():
    global _NC
    if _NC is None:
        _NC = _build()
    return _NC


def _gblk():
    g = np.zeros((128, 8), dtype=np.float32)
    for p in range(128):
        g[p, p // CPG] = 1.0
    return g


def kernel(x, gn_gamma, gn_beta, w_qkv, w_out, b_out, trace=False):
    x = np.ascontiguousarray(np.asarray(x, dtype=np.float32))
    wqkvT = np.ascontiguousarray(np.asarray(w_qkv, np.float32).T)
    woutT = np.ascontiguousarray(np.asarray(w_out, np.float32).T)
    gbo = np.ascontiguousarray(np.stack(
        [np.asarray(gn_gamma, np.float32).reshape(C),
         np.asarray(gn_beta, np.float32).reshape(C),
         np.asarray(b_out, np.float32).reshape(C)], axis=1))
    gblk = _gblk()
    gbt = np.ascontiguousarray(gblk.T)

    nc = _get_nc()
    in_maps = []
    for core in range(8):
        b, half = core // 2, core % 2
        in_maps.append({
            "x": x[b],
            "xq": np.ascontiguousarray(x[b][:, half * NQ:(half + 1) * NQ]),
            "wqkvT": wqkvT,
            "woutT": woutT,
            "gbo": gbo,
            "gblk": gblk,
            "gbt": gbt,
        })
    res = run_bass_kernel_spmd(nc, in_maps, core_ids=list(range(8)),
                               trace=trace)
    y = np.empty((B, C, N), dtype=np.float32)
    for core in range(8):
        b, half = core // 2, core % 2
        y[b][:, half * NQ:(half + 1) * NQ] = res.results[core]["y"]
    if trace:
        kernel.last_results = res
    return y


# revision 20
# speedup vs baseline: 1.0003x; 1.0003x over previous
"""EnhancedTemporalAttention Trainium2 kernel.

Full module: GroupNorm(32) -> QKV 1x1conv -> 8-head attention (softmax) ->
out 1x1conv + bias -> +residual, on x [4, 512, 2048] fp32.

Sharding: 8 cores = (batch b = core//2) x (query half = core%2).  Each core
computes GroupNorm + K/V projections over the full sequence for its batch
(duplicated across the pair), Q projection + attention + out projection for
its 1024-query half.  Output slices are disjoint; host just concatenates.

All matmuls run as float32r (fp32 storage, reduced-precision multiply at
full PE rate).  Attention uses the transposed-scores layout (keys on
partitions); softmax denominators ride as a 65th ones-row on the V^T
stationary operand; exp runs on ScalarE straight out of PSUM.  The first
head-pair's attention is interleaved with the V^T projection so ScalarE
starts its exp stream early.
"""
import sys

sys.path.insert(0, "/opt/trn_rl_repo")

import numpy as np

import concourse.bacc as bacc
import concourse.bass as bass
import concourse.tile as tile
from concourse import mybir
from concourse.bass_utils import run_bass_kernel_spmd

F32 = mybir.dt.float32
F32R = mybir.dt.float32r

B = 4
C = 512
N = 2048
NQ = 1024          # queries per core
H = 8
D = 64
G = 32             # groupnorm groups
CPG = C // G       # 16 channels per group
EPS = 1e-4
SCALE = D ** -0.5
NT = C // 128      # 4 channel tiles
NKB = N // 128     # 16 key blocks
AF = mybir.ActivationFunctionType
ALU = mybir.AluOpType

# (j, h) chunk sequence in groups of 3 (one exp instruction each)
CHUNKS = [(j, h) for j in range(NKB) for h in range(2)]
GROUPS = [CHUNKS[i:i + 3] for i in range(0, len(CHUNKS), 3)]


def _build(taps=False):
    nc = bacc.Bacc("TRN2", target_bir_lowering=False, debug=False)
    x_in = nc.dram_tensor("x", [C, N], F32, kind="ExternalInput").ap()
    xq_in = nc.dram_tensor("xq", [C, NQ], F32, kind="ExternalInput").ap()
    wqkvT_in = nc.dram_tensor("wqkvT", [C, 3 * C], F32, kind="ExternalInput").ap()
    woutT_in = nc.dram_tensor("woutT", [C, C], F32, kind="ExternalInput").ap()
    gbo_in = nc.dram_tensor("gbo", [C, 3], F32, kind="ExternalInput").ap()
    gblk_in = nc.dram_tensor("gblk", [128, 8], F32, kind="ExternalInput").ap()
    gbt_in = nc.dram_tensor("gbt", [8, 128], F32, kind="ExternalInput").ap()
    y_out = nc.dram_tensor("y", [C, NQ], F32, kind="ExternalOutput").ap()
    # scratch
    kind_t = "ExternalOutput" if taps else "Internal"
    mr_d = nc.dram_tensor("mr_d", [C, 2], F32, kind=kind_t).ap()
    den_d = nc.dram_tensor("den_d", [H * NQ], F32, kind=kind_t).ap()
    den2_d = nc.dram_tensor("den2_d", [H * NQ], F32).ap()
    tap = {}
    if taps:
        for nm, shp in (("t_xn0", [128, N]), ("t_o65a", [128, 512]),
                        ("t_o65b", [128, 512]), ("t_op00", [128, 512])):
            tap[nm] = nc.dram_tensor(nm, shp, F32, kind="ExternalOutput").ap()

    from contextlib import ExitStack
    with tile.TileContext(nc) as tc, ExitStack() as ctx:
        persist = ctx.enter_context(tc.tile_pool(name="persist", bufs=1))
        gn = ctx.enter_context(tc.tile_pool(name="gn", bufs=1))
        pspool = ctx.enter_context(tc.tile_pool(name="ps", bufs=1,
                                                space="PSUM"))
        expp = ctx.enter_context(tc.tile_pool(name="expp", bufs=3))
        o65p = ctx.enter_context(tc.tile_pool(name="o65p", bufs=6))

        # persistent activation tensors
        q_sb = [persist.tile([128, NQ], F32R, tag=f"q{m}", name=f"q{m}")
                for m in range(NT)]
        k_sb = [persist.tile([128, N], F32R, tag=f"k{m}", name=f"k{m}")
                for m in range(NT)]
        vT_sb = [persist.tile([128, H, D + 1], F32R, tag=f"vT{nb}",
                              name=f"vT{nb}") for nb in range(NKB)]

        den_r = den_d.rearrange("(a b) -> a b", b=512)

        def new_S():
            return pspool.tile([128, 3, 512], F32, tag="S", name="S",
                               bufs=2)

        def attn_group(qc, m, av, grp_chunks):
            ng = len(grp_chunks)
            psc = new_S()
            for i, (j, h) in enumerate(grp_chunks):
                nc.tensor.matmul(
                    psc[:, i, :],
                    lhsT=k_sb[m][h * D:(h + 1) * D, j * 128:(j + 1) * 128],
                    rhs=q_sb[m][h * D:(h + 1) * D,
                                qc * 512:(qc + 1) * 512],
                    start=True, stop=True, tile_position=(h * D, 0))
            eT = expp.tile([128, 3, 512], F32R, tag="e", name="e")
            nc.scalar.activation(out=eT[:, 0:ng, :], in_=psc[:, 0:ng, :],
                                 func=AF.Exp, scale=SCALE)
            for i, (j, h) in enumerate(grp_chunks):
                nc.tensor.matmul(
                    av[h], lhsT=vT_sb[j][:, 2 * m + h, :], rhs=eT[:, i, :],
                    start=(j == 0), stop=(j == NKB - 1))

        def pair_drain(qc, m, av):
            o65s = []
            for h in range(2):
                o65 = o65p.tile([128, 512], F32, tag="o65", name="o65")
                nc.vector.tensor_copy(o65[0:D + 1, :], av[h][0:D + 1, :])
                if taps and m == 0 and qc == 0:
                    nc.sync.dma_start(
                        out=tap["t_o65a" if h == 0 else "t_o65b"], in_=o65)
                nc.scalar.dma_start(out=den_r[qc * 8 + m * 2 + h, :],
                                    in_=o65[D:D + 1, :])
                o65s.append(o65)
            return o65s

        with tc.tile_pool(name="xpool", bufs=1) as xpool, \
             tc.tile_pool(name="xnpool", bufs=1) as xnpool, \
             tc.tile_pool(name="wq", bufs=1) as wqp:
            # ---- input loads, critical-path first: x (in 512-col chunks
            # so bn_stats pipelines), then qkv weights ----
            X = []    # f32r storage; XF = f32 views for DVE reads
            for t in range(NT):
                xt = xpool.tile([128, N], F32R, tag=f"X{t}", name=f"X{t}")
                for sg in range(4):
                    nc.sync.dma_start(
                        out=xt[:, sg * 512:(sg + 1) * 512],
                        in_=x_in[t * 128:(t + 1) * 128,
                                 sg * 512:(sg + 1) * 512].bitcast(F32R))
                X.append(xt)
            XF = [xt.bitcast(F32) for xt in X]
            # ACT table preload off the critical path (Sqrt now; Exp is
            # chained after the real Sqrt below so it can't evict it early)
            eps_t = gn.tile([G, 1], F32, tag="eps_t")
            nc.vector.memset(eps_t, EPS)
            sqw = gn.tile([G, 1], F32, tag="sqw")
            nc.scalar.activation(out=sqw, in_=eps_t, func=AF.Sqrt)
            gblk = gn.tile([128, 8], F32R, tag="gblk")
            nc.sync.dma_start(out=gblk, in_=gblk_in.bitcast(F32R))
            gbt = gn.tile([8, 128], F32R, tag="gbt")
            nc.sync.dma_start(out=gbt, in_=gbt_in.bitcast(F32R))
            XQ = []
            gbo = []
            for t in range(NT):
                xqt = persist.tile([128, NQ], F32, tag=f"XQ{t}",
                                   name=f"XQ{t}")
                nc.sync.dma_start(out=xqt,
                                  in_=xq_in[t * 128:(t + 1) * 128, :])
                XQ.append(xqt)
                gt = persist.tile([128, 3], F32, tag=f"gbo{t}",
                                  name=f"gbo{t}")
                nc.sync.dma_start(out=gt,
                                  in_=gbo_in[t * 128:(t + 1) * 128, :])
                gbo.append(gt)
            wT = [wqp.tile([128, 3 * C], F32R, tag=f"wT{kc}",
                           name=f"wT{kc}") for kc in range(NT)]
            for sl in (1, 0, 2):   # k first (k-proj is emitted first)
                for kc in range(NT):
                    nc.sync.dma_start(
                        out=wT[kc][:, sl * C:(sl + 1) * C],
                        in_=wqkvT_in[kc * 128:(kc + 1) * 128,
                                     sl * C:(sl + 1) * C].bitcast(F32R))

            # ---- GroupNorm stats: bn_stats -> per-channel (mean, E[x^2])
            # -> PE block-ones matmul reduces 16-channel groups ----
            mvv = []
            for t in range(NT):
                stats = gn.tile([128, 4, 6], F32, tag=f"st{t}",
                                name=f"st{t}")
                for sg in range(4):
                    nc.vector.bn_stats(out=stats[:, sg, :],
                                       in_=XF[t][:, sg * 512:(sg + 1) * 512])
                mv = gn.tile([128, 2], F32, tag=f"mv{t}", name=f"mv{t}")
                nc.vector.bn_aggr(out=mv, in_=stats)
                mt = gn.tile([128, 2], F32R, tag=f"mvv{t}", name=f"mvv{t}")
                nc.vector.tensor_copy(mt[:, 0:1], mv[:, 0:1])
                sqm = gn.tile([128, 1], F32, tag=f"sqm{t}", name=f"sqm{t}")
                # E[x^2] = var + mean^2
                nc.vector.tensor_mul(sqm, mv[:, 0:1], mv[:, 0:1])
                nc.vector.tensor_tensor(out=mt[:, 1:2], in0=mv[:, 1:2],
                                        in1=sqm, op=ALU.add)
                mvv.append(mt)
            g8ps = new_S()     # group sums land in psum bank 0, [8, 8]
            for t in range(NT):
                nc.tensor.matmul(g8ps[0:8, 0, t * 2:(t + 1) * 2],
                                 lhsT=gblk, rhs=mvv[t],
                                 start=(t == 0), stop=(t == NT - 1),
                                 skip_group_check=True)
            g8 = gn.tile([8, NT, 2], F32, tag="g8")
            nc.vector.tensor_copy(g8.rearrange("p t s -> p (t s)"),
                                  g8ps[0:8, 0, 0:8])
            mean8 = gn.tile([8, NT], F32, tag="mean8")
            nc.vector.tensor_scalar_mul(mean8, g8[:, :, 0], 1.0 / CPG)
            ex28 = gn.tile([8, NT], F32, tag="ex28")
            nc.vector.tensor_scalar_mul(ex28, g8[:, :, 1], 1.0 / CPG)
            msq8 = gn.tile([8, NT], F32, tag="msq8")
            nc.vector.tensor_mul(msq8, mean8, mean8)
            var8 = gn.tile([8, NT], F32, tag="var8")
            nc.vector.tensor_tensor(out=var8, in0=ex28, in1=msq8,
                                    op=ALU.subtract)
            std8 = gn.tile([8, NT], F32, tag="std8")
            nc.scalar.activation(out=std8, in_=var8, func=AF.Sqrt,
                                 bias=eps_t[0:8, :])
            rstd8 = gn.tile([8, NT], F32, tag="rstd8")
            nc.vector.reciprocal(rstd8, std8)
            # preload the Exp table now; input std8 forces it after Sqrt
            warm = gn.tile([8, NT], F32, tag="warm")
            nc.scalar.activation(out=warm, in_=std8, func=AF.Exp)
            mr8 = gn.tile([8, NT, 2], F32R, tag="mr8")
            nc.vector.tensor_copy(mr8[:, :, 0:1],
                                  mean8.rearrange("p (t o) -> p t o", o=1))
            nc.vector.tensor_copy(mr8[:, :, 1:2],
                                  rstd8.rearrange("p (t o) -> p t o", o=1))
            # broadcast group stats to channels via a K=8 ones matmul
            msps = new_S()
            for t in range(NT):
                nc.tensor.matmul(msps[:, 0, t * 2:(t + 1) * 2],
                                 lhsT=gbt, rhs=mr8[:, t, :],
                                 start=(t == 0), stop=(t == NT - 1),
                                 skip_group_check=True)
            mscall = msps[:, 0, 0:2 * NT].rearrange("p (t s) -> p t s", s=2)

            # per-channel scale/bias, then normalize (in place over X)
            xnq = []
            for t in range(NT):
                scale_c = gn.tile([128, 1], F32, tag=f"sc{t}", name=f"sc{t}")
                nc.vector.tensor_mul(scale_c, mscall[:, t, 1:2],
                                     gbo[t][:, 0:1])
                tmp = gn.tile([128, 1], F32, tag=f"tmp{t}", name=f"tmp{t}")
                nc.vector.tensor_mul(tmp, mscall[:, t, 0:1], scale_c)
                bias_c = gn.tile([128, 1], F32, tag=f"bc{t}", name=f"bc{t}")
                nc.vector.tensor_tensor(out=bias_c, in0=gbo[t][:, 1:2],
                                        in1=tmp, op=ALU.subtract)
                xnt = X[t]
                nc.vector.tensor_scalar(out=xnt, in0=XF[t], scalar1=scale_c,
                                        scalar2=bias_c, op0=ALU.mult,
                                        op1=ALU.add)
                if taps and t == 0:
                    nc.sync.dma_start(out=tap["t_xn0"], in_=xnt.bitcast(F32))
                xnqt = xnpool.tile([128, NQ], F32R, tag=f"xnq{t}",
                                   name=f"xnq{t}")
                nc.vector.tensor_scalar(out=xnqt, in0=XQ[t], scalar1=scale_c,
                                        scalar2=bias_c, op0=ALU.mult,
                                        op1=ALU.add)
                xnq.append(xnqt)
            xn = X

            # ---------- QKV projections ----------
            ones_sb = gn.tile([128, H], F32, tag="ones_sb")
            nc.vector.memset(ones_sb, 1.0)
            # k: rows 512..1023, over full sequence
            for m in range(NT):
                for ncx in range(N // 512):
                    psy = new_S()
                    ps = psy[:, 0, :]
                    for kc in range(NT):
                        nc.tensor.matmul(
                            ps, lhsT=wT[kc][:, C + m * 128:C + (m + 1) * 128],
                            rhs=xn[kc][:, ncx * 512:(ncx + 1) * 512],
                            start=(kc == 0), stop=(kc == NT - 1))
                    nc.vector.tensor_copy(
                        k_sb[m][:, ncx * 512:(ncx + 1) * 512], ps)
            # q: rows 0..511 of qkv, over the core's query half
            for m in range(NT):
                for ncx in range(NQ // 512):
                    psy = new_S()
                    ps = psy[:, 0, :]
                    for kc in range(NT):
                        nc.tensor.matmul(
                            ps, lhsT=wT[kc][:, m * 128:(m + 1) * 128],
                            rhs=xnq[kc][:, ncx * 512:(ncx + 1) * 512],
                            start=(kc == 0), stop=(kc == NT - 1))
                    nc.vector.tensor_copy(
                        q_sb[m][:, ncx * 512:(ncx + 1) * 512], ps)
            # v^T interleaved with head-pair (0,0) attention so ScalarE's
            # exp stream starts while PE is still projecting
            av00 = [pspool.tile([D + 1, 512], F32, tag="av", name="av",
                                bufs=2) for _ in range(2)]
            started = [False]
            gi_by_maxj = {}
            for gi, gch in enumerate(GROUPS):
                mj = max(j for j, _ in gch)
                gi_by_maxj.setdefault(mj, []).append(gi)
            for nb in range(NKB):
                psy = new_S()
                ps = psy[:, 0, :]
                for kc in range(NT):
                    nc.tensor.matmul(
                        ps, lhsT=xn[kc][:, nb * 128:(nb + 1) * 128],
                        rhs=wT[kc][:, 2 * C:3 * C],
                        start=(kc == 0), stop=(kc == NT - 1))
                nc.vector.tensor_copy(
                    vT_sb[nb][:, :, 0:D],
                    ps.rearrange("p (h d) -> p h d", h=H))
                nc.vector.tensor_copy(
                    vT_sb[nb][:, :, D:D + 1],
                    ones_sb.rearrange("p (h o) -> p h o", o=1))
                for gi in gi_by_maxj.get(nb, []):
                    attn_group(0, 0, av00, GROUPS[gi])
            o65s00 = pair_drain(0, 0, av00)

        # ---------- rest of attention ----------
        with tc.tile_pool(name="opp", bufs=1) as opp, \
             tc.tile_pool(name="ytp", bufs=3) as ytp, \
             tc.tile_pool(name="rbp", bufs=6) as rbp:
            opair = {(kc, nck): opp.tile([128, 512], F32R,
                                         tag=f"op{kc}_{nck}",
                                         name=f"op{kc}_{nck}")
                     for kc in range(NT) for nck in range(2)}
            woutT = []
            for t in range(NT):
                wt = opp.tile([128, C], F32R, tag=f"woT{t}", name=f"woT{t}")
                nc.sync.dma_start(
                    out=wt,
                    in_=woutT_in[t * 128:(t + 1) * 128, :].bitcast(F32R))
                woutT.append(wt)

            def do_pair(qc, m):
                av = [pspool.tile([D + 1, 512], F32, tag="av", name="av",
                                  bufs=2) for _ in range(2)]
                for gch in GROUPS:
                    attn_group(qc, m, av, gch)
                return pair_drain(qc, m, av)

            def do_norm(qc, m, o65s):
                """reciprocal of the pair's dens (repacked [128,8]),
                broadcast back, normalize into opair[(m, qc)]."""
                r0 = qc * 8 + m * 2
                dpack = rbp.tile([128, 8], F32, tag="dpack", name="dpack")
                nc.scalar.dma_start(
                    out=dpack,
                    in_=den_d.rearrange("(a b) -> a b", b=8)[
                        r0 * 64:(r0 + 2) * 64, :])
                nc.vector.reciprocal(dpack, dpack)
                nc.scalar.dma_start(
                    out=den2_d.rearrange("(a b) -> a b", b=8)[
                        r0 * 64:(r0 + 2) * 64, :],
                    in_=dpack)
                op = opair[(m, qc)]
                # head 0: rb at partitions 0..63, multiply into opair rows
                rb = rbp.tile([D, 512], F32, tag="rb", name="rb")
                nc.scalar.dma_start(
                    out=rb,
                    in_=bass.AP(tensor=den2_d.tensor,
                                offset=den2_d.offset + r0 * 512,
                                ap=[[0, D], [1, 512]]))
                nc.vector.tensor_mul(op[0:D, :], o65s[0][0:D, :], rb)
                # head 1: shift raw rows into opair[64:128] while loading
                # its reciprocal bcast at partitions 64..127, then multiply
                # in place
                rb2 = rbp.tile([128, 512], F32, tag="rb2", name="rb2")
                nc.scalar.dma_start(
                    out=rb2[D:2 * D, :],
                    in_=bass.AP(tensor=den2_d.tensor,
                                offset=den2_d.offset + (r0 + 1) * 512,
                                ap=[[0, D], [1, 512]]))
                nc.scalar.dma_start(out=op[D:2 * D, :],
                                    in_=o65s[1][0:D, :].bitcast(F32R))
                nc.vector.tensor_mul(op[D:2 * D, :],
                                     op[D:2 * D, :].bitcast(F32),
                                     rb2[D:2 * D, :])

            def do_outproj(qc):
                for m2 in range(NT):
                    psy = new_S()
                    ps = psy[:, 0, :]
                    for kc in range(NT):
                        nc.tensor.matmul(
                            ps, lhsT=woutT[kc][:, m2 * 128:(m2 + 1) * 128],
                            rhs=opair[(kc, qc)],
                            start=(kc == 0), stop=(kc == NT - 1))
                    yt = ytp.tile([128, 512], F32, tag="yt", name="yt")
                    nc.vector.scalar_tensor_tensor(
                        out=yt, in0=ps, scalar=gbo[m2][:, 2:3],
                        in1=XQ[m2][:, qc * 512:(qc + 1) * 512],
                        op0=ALU.add, op1=ALU.add)
                    nc.sync.dma_start(
                        out=y_out[m2 * 128:(m2 + 1) * 128,
                                  qc * 512:(qc + 1) * 512],
                        in_=yt)

            # software-pipelined emission: out-proj of half 0 lands in the
            # middle of half 1's attention stream
            do_norm(0, 0, o65s00)
            for m in range(1, NT):
                do_norm(0, m, do_pair(0, m))
            for m in range(2):
                do_norm(1, m, do_pair(1, m))
            do_outproj(0)
            if taps:
                nc.sync.dma_start(out=tap["t_op00"],
                                  in_=opair[(0, 0)].bitcast(F32))
            for m in range(2, NT):
                do_norm(1, m, do_pair(1, m))
            do_outproj(1)

    nc.compile()
    return nc


_NC = None


def _get_nc():
    global _NC
    if _NC is None:
        _NC = _build()
    return _NC


def _gblk():
    g = np.zeros((128, 8), dtype=np.float32)
    for p in range(128):
        g[p, p // CPG] = 1.0
    return g


def kernel(x, gn_gamma, gn_beta, w_qkv, w_out, b_out, trace=False):
    x = np.ascontiguousarray(np.asarray(x, dtype=np.float32))
    wqkvT = np.ascontiguousarray(np.asarray(w_qkv, np.float32).T)
    woutT = np.ascontiguousarray(np.asarray(w_out, np.float32).T)
    gbo = np.ascontiguousarray(np.stack(
        [np.asarray(gn_gamma, np.float32).reshape(C),
         np.asarray(gn_beta, np.float32).reshape(C),
         np.asarray(b_out, np.float32).reshape(C)], axis=1))
    gblk = _gblk()
    gbt = np.ascontiguousarray(gblk.T)

    nc = _get_nc()
    in_maps = []
    for core in range(8):
        b, half = core // 2, core % 2
        in_maps.append({
            "x": x[b],
            "xq": np.ascontiguousarray(x[b][:, half * NQ:(half + 1) * NQ]),
            "wqkvT": wqkvT,
            "woutT": woutT,
            "gbo": gbo,
            "gblk": gblk,
            "gbt": gbt,
        })
    res = run_bass_kernel_spmd(nc, in_maps, core_ids=list(range(8)),
                               trace=trace)
    y = np.empty((B, C, N), dtype=np.float32)
    for core in range(8):
        b, half = core // 2, core % 2
        y[b][:, half * NQ:(half + 1) * NQ] = res.results[core]["y"]
    if trace:
        kernel.last_results = res
    return y
